# revision 21
# baseline (speedup 1.0000x reference)
"""Causal self-attention Bass kernel for Trainium2, 8 NeuronCores.

Problem shapes (hardcoded): x (4, 2048, 512), W_qkv (1536, 512),
W_out (512, 512), b_out (512,); NH=8 heads, DH=64.

Sharding: core c handles batch b = c // 2 and head group g = c % 2
(4 heads = 2 head-pairs each). Host sums the two partial output
projections per batch and adds the bias.

Design notes (engine cost model, measured):
  - ACT exp is the hard floor: causal att cols / 1.2 GHz ~ 58 us/core;
    ACT also has a ~352-cycle fixed cost per ACTIVATE. The kernel is
    built to keep ACT ~fully busy and everything else under it.
  - PE work is halved via tile_position packing: with 64-token k-chunks
    a head-PAIR stacks on partition halves (A rows 0:64, B rows 64:128)
    so scores (K=DH=64) run as concurrent row+col-tiled matmul pairs
    (A at (0,0), B at (64,64)) and att@v (K=64 k-tokens) as concurrent
    row-tiled pairs (A (0,0) -> ps_yA, B (64,0) -> ps_yB).
  - Attention is q-outer (flash style): per (pair, 512-col q-quarter),
    accumulate y over k-chunks j = 0..8c+7 into [65, 512] PSUM tiles
    (ones-augmented v gives the softmax denominator for free), then
    normalize that quarter while later quarters proceed.
  - Scores live in a manual 4-slot PSUM ring [128, 4, 512]; exp reads
    two adjacent slots per ACTIVATE ([128, <=1024]) directly from PSUM
    (ACT is the cheapest PSUM->SBUF mover) into small rotating att
    tiles; av consumes att at a 2-iteration lag so exp latency stays
    off the PE critical path.
  - Causal masking: fills/avs restrict to valid q-cols (geometry); the
    64-wide diagonal triangle is masked post-exp on the idle gpsimd
    engine (triu stacked twice vertically covers both heads at once).
    Garbage PSUM cols under the diagonal are exp'd but never consumed.
  - PSUM budget: ring 8KB + 2x2 quarter accumulators 8KB = 16KB (full).
  - Output projection is spread across pair-1 quarters; bf16 output
    halves the store DMA; host adds bias in f32.
"""

import sys

if "/opt/trn_rl_repo" not in sys.path:
    sys.path.insert(0, "/opt/trn_rl_repo")

import numpy as np
import ml_dtypes

B, T, D, NH, DH = 4, 2048, 512, 8, 64
HPC = 4  # heads per core
N_CORES = 8
BF16 = ml_dtypes.bfloat16

_PROG = None


def _build_program():
    import concourse.bass as bass
    import concourse.mybir as mybir
    import concourse.tile as tile
    from concourse import bacc

    f32 = mybir.dt.float32
    bf16 = mybir.dt.bfloat16
    Exp = mybir.ActivationFunctionType.Exp

    nc = bacc.Bacc("TRN2", target_bir_lowering=False, debug=False)

    xT_d = nc.dram_tensor("xT", [D, T], bf16, kind="ExternalInput").ap()
    wqkT_d = nc.dram_tensor("wqkT", [D, 512], bf16, kind="ExternalInput").ap()
    wvT_d = nc.dram_tensor("wvT", [D, 256], bf16, kind="ExternalInput").ap()
    woT_d = nc.dram_tensor("woT", [256, D], bf16, kind="ExternalInput").ap()
    triu_d = nc.dram_tensor("triu2", [128, 64], bf16, kind="ExternalInput").ap()
    out_d = nc.dram_tensor("out", [T, D], bf16, kind="ExternalOutput").ap()

    NJ = T // 64  # 32 k-chunks of 64 tokens

    with tile.TileContext(nc) as tc:
        with (
            tc.tile_pool(name="const", bufs=1) as cpool,
            tc.tile_pool(name="att", bufs=2) as apool,
            tc.tile_pool(name="work", bufs=2) as wpool,
            tc.tile_pool(name="outsb", bufs=2) as opool,
            tc.tile_pool(name="ps", bufs=1, space="PSUM") as pspool,
        ):
            # ---- input DMAs ----
            xT_sb = cpool.tile([128, 4, T], bf16, tag="xT")
            for kc in range(4):
                nc.sync.dma_start(out=xT_sb[:, kc, :], in_=xT_d[128 * kc : 128 * (kc + 1), :])
            wqkT_sb = cpool.tile([128, 4, 512], bf16, tag="wqkT")
            for kc in range(4):
                nc.sync.dma_start(out=wqkT_sb[:, kc, :], in_=wqkT_d[128 * kc : 128 * (kc + 1), :])
            wvT_sb = cpool.tile([128, 4, 256], bf16, tag="wvT")
            for kc in range(4):
                nc.sync.dma_start(out=wvT_sb[:, kc, :], in_=wvT_d[128 * kc : 128 * (kc + 1), :])
            woT_sb = cpool.tile([128, 2, 512], bf16, tag="woT")
            for kc in range(2):
                nc.sync.dma_start(out=woT_sb[:, kc, :], in_=woT_d[128 * kc : 128 * (kc + 1), :])
            triu_sb = cpool.tile([128, 64], bf16, tag="triu2")
            nc.sync.dma_start(out=triu_sb[:, :], in_=triu_d[:, :])

            # qkT chunks: [qT-pair0 | kT-pair0 | qT-pair1 | kT-pair1];
            # head 2p+b occupies partitions 64b..64b+64 of chunks 2p, 2p+1.
            qkT_sb = cpool.tile([128, 4, T], bf16, tag="qkT")
            # v_all[64b + t, j2, par, hp, 0:64] = v of head 2hp+b, k-token
            # 64*(2*j2+par) + t; column 64 is the ones column (denominator
            # trick). The (j2, par) split keeps the v-proj scatter APs
            # contiguous (partition half sg of a 128-token chunk lands on
            # chunk parity sg).
            v_all = cpool.tile([128, NJ // 2, 2, 2, 65], bf16, tag="v_all")
            ynormT = cpool.tile([128, 2, T], bf16, tag="ynormT")

            nc.gpsimd.memset(v_all[:, :, :, :, 64:65], 1.0)
            # all-ones; row 0 is the lhsT of the rank-1 den broadcast
            ones64 = cpool.tile([65, 64], f32, tag="ones64")
            nc.gpsimd.memset(ones64[:, :], 1.0)

            # ---- PSUM ----
            # scores ring: 4 slots of [128, 512]; exp covers 2 adjacent
            ring = pspool.tile([128, 4, 512], f32, tag="ring")
            ps_ya = [
                pspool.tile([65, 512], f32, tag="ps_ya", bufs=2, name=f"ps_ya{i}")
                for i in range(2)
            ]
            ps_yb = [
                pspool.tile([65, 512], f32, tag="ps_yb", bufs=2, name=f"ps_yb{i}")
                for i in range(2)
            ]
            slot_ctr = [0]

            def next_slots(n):
                # n consecutive ring slots, aligned, no wrap (n in {1,2})
                s = slot_ctr[0]
                if s % n:
                    s += n - (s % n)
                if s % 4 + n > 4:
                    s += 4 - s % 4
                slot_ctr[0] = s + n
                return s % 4

            # ---- phase A: projections ----
            def emit_proj_m(m):
                for half in range(2):
                    s = next_slots(2)
                    ps = ring[:, s : s + 2, :]
                    for n2 in range(2):
                        n = 2 * half + n2
                        for kc in range(4):
                            nc.tensor.matmul(
                                ps[:, n2, :],
                                lhsT=wqkT_sb[:, kc, 128 * m : 128 * (m + 1)],
                                rhs=xT_sb[:, kc, 512 * n : 512 * (n + 1)],
                                start=(kc == 0),
                                stop=(kc == 3),
                            )
                    nc.vector.tensor_copy(
                        qkT_sb[:, m, 1024 * half : 1024 * (half + 1)],
                        ps.rearrange("p a b -> p (a b)"),
                    )

            def emit_proj_v(quarter):
                s = next_slots(2)
                ps = ring[:, s : s + 2, :]
                for tl in range(4):
                    t = 4 * quarter + tl
                    for kc in range(4):
                        nc.tensor.matmul(
                            ps[:, tl // 2, 256 * (tl % 2) : 256 * (tl % 2 + 1)],
                            lhsT=xT_sb[:, kc, 128 * t : 128 * (t + 1)],
                            rhs=wvT_sb[:, kc, :],
                            start=(kc == 0),
                            stop=(kc == 3),
                        )
                # scatter [128 t, 4 heads x 64] into the A|B stacked layout;
                # slot cols = (u: 128-token half, hp, b, e); token =
                # 512q + 128*(2a+u) + pt, so j2 = 4q + 2a + u, par = pt//64
                src = ps.rearrange("p a (u hp b e) -> p (a u) hp b e", u=2, hp=2, b=2)
                for sg in range(2):  # src partition half -> chunk parity
                    for b in range(2):  # head-of-pair
                        dst = v_all[
                            64 * b : 64 * (b + 1),
                            4 * quarter : 4 * quarter + 4,
                            sg,
                            :,
                            0:64,
                        ]
                        nc.vector.tensor_copy(
                            dst, src[64 * sg : 64 * (sg + 1), :, :, b, :]
                        )

            emit_proj_m(0)
            emit_proj_v(0)
            emit_proj_m(1)
            emit_proj_v(1)
            emit_proj_m(2)
            emit_proj_v(2)
            emit_proj_m(3)
            emit_proj_v(3)

            # ---- phases B/C/D: attention, q-outer ----
            def emit_fill(p, c, j, s):
                """packed scores of head-pair p, k-chunk j, quarter c
                into ring slot s (A -> partitions 0:64, B -> 64:128)."""
                d = max(0, 64 * j - 512 * c)
                for b in range(2):
                    nc.tensor.matmul(
                        ring[64 * b : 64 * (b + 1), s, d:512],
                        lhsT=qkT_sb[64 * b : 64 * (b + 1), 2 * p + 1, 64 * j : 64 * (j + 1)],
                        rhs=qkT_sb[64 * b : 64 * (b + 1), 2 * p, 512 * c + d : 512 * (c + 1)],
                        start=True,
                        stop=True,
                        tile_position=(64 * b, 64 * b),
                    )

            def emit_av(p, c, j, att, off, last_j):
                d = max(0, 64 * j - 512 * c)
                ya, yb = ps_ya[c % 2], ps_yb[c % 2]
                for b, ps_y in ((0, ya), (1, yb)):
                    nc.tensor.matmul(
                        ps_y[:, d:512],
                        lhsT=v_all[64 * b : 64 * (b + 1), j // 2, j % 2, p, :],
                        rhs=att[64 * b : 64 * (b + 1), off + d : off + 512],
                        start=(j == 0),
                        stop=(j == last_j),
                        tile_position=(64 * b, 0),
                    )

            def emit_norm(p, c, b, ps_y):
                h = 2 * p + b
                den = wpool.tile([1, 512], f32, tag="den", name=f"den{h}_{c}")
                nc.vector.tensor_copy(den[:, :], ps_y[64:65, :])
                s = next_slots(1)
                nc.tensor.matmul(
                    ring[0:64, s, :],
                    lhsT=ones64[0:1, :],
                    rhs=den[0:1, :],
                    start=True,
                    stop=True,
                )
                recb = wpool.tile([64, 512], f32, tag="recb", name=f"recb{h}_{c}")
                nc.vector.reciprocal_approx_fast(out=recb[:, :], in_=ring[0:64, s, :])
                dst = ynormT[64 * b : 64 * (b + 1), p, 512 * c : 512 * (c + 1)]
                nc.vector.tensor_mul(dst, ps_y[0:64, :], recb[:, :])

            def emit_outproj(grp):
                s = next_slots(2)
                ps = ring[:, s : s + 2, :]
                for i2 in range(2):
                    i = 2 * grp + i2
                    for kc in range(2):
                        nc.tensor.matmul(
                            ps[:, i2, :],
                            lhsT=ynormT[:, kc, 128 * i : 128 * (i + 1)],
                            rhs=woT_sb[:, kc, :],
                            start=(kc == 0),
                            stop=(kc == 1),
                        )
                osb = opool.tile([128, 1024], bf16, tag="osb")
                nc.vector.tensor_copy(osb[:, :], ps.rearrange("p a b -> p (a b)"))
                for i2 in range(2):
                    i = 2 * grp + i2
                    nc.sync.dma_start(
                        out=out_d[128 * i : 128 * (i + 1), :],
                        in_=osb[:, 512 * i2 : 512 * i2 + 512],
                    )

            for p in range(2):
                for c in range(4):
                    njs = 8 * c + 8
                    last_j = njs - 1
                    pend = []  # (j, att, off) awaiting av
                    att = None
                    s0 = None
                    for n in range(njs):
                        j = n
                        if n % 2 == 0:
                            s0 = next_slots(2)
                            att = apool.tile(
                                [128, 1024], bf16, tag="att", bufs=4,
                                name=f"att{p}_{c}_{n}",
                            )
                            emit_fill(p, c, j, s0)
                            pend.append((j, att, 0))
                        else:
                            emit_fill(p, c, j, s0 + 1)
                            pend.append((j, att, 512))
                            # exp over the slot pair; skip leading garbage
                            # cols of the even slot (diagonal deficit)
                            d1 = max(0, 64 * (j - 1) - 512 * c)
                            nc.scalar.activation(
                                att[:, d1:1024],
                                ring[:, s0 : s0 + 2, :].rearrange(
                                    "p a b -> p (a b)"
                                )[:, d1:1024],
                                Exp,
                                scale=0.125,
                            )
                            # post-exp triangular mask on diagonal blocks
                            for jj, at, off in pend[-2:]:
                                dd = 64 * jj - 512 * c
                                if dd >= 0:
                                    nc.gpsimd.tensor_mul(
                                        at[:, off + dd : off + dd + 64],
                                        at[:, off + dd : off + dd + 64],
                                        triu_sb[:, :],
                                    )
                        if n >= 2:
                            jj, at, off = pend[n - 2]
                            emit_av(p, c, jj, at, off, last_j)
                    for jj, at, off in pend[njs - 2 :]:
                        emit_av(p, c, jj, at, off, last_j)
                    emit_norm(p, c, 0, ps_ya[c % 2])
                    emit_norm(p, c, 1, ps_yb[c % 2])
                    if p == 1:
                        emit_outproj(2 * c)
                        emit_outproj(2 * c + 1)

    nc.compile()
    return nc


def _get_program():
    global _PROG
    if _PROG is None:
        _PROG = _build_program()
    return _PROG


def _make_in_maps(x, W_qkv, W_out):
    in_maps = []
    tri = np.triu(np.ones((64, 64), np.float32))
    triu2 = np.concatenate([tri, tri], axis=0).astype(BF16)  # (128, 64)
    for c in range(N_CORES):
        b, g = c // 2, c % 2
        heads = [4 * g + i for i in range(HPC)]
        xT = np.ascontiguousarray(x[b].T).astype(BF16)
        # qkT chunk order: [q-pair0, k-pair0, q-pair1, k-pair1]
        rows = []
        for hp in range(2):
            h0, h1 = heads[2 * hp], heads[2 * hp + 1]
            rows.append(W_qkv[64 * h0 : 64 * h0 + 64])  # q of h0
            rows.append(W_qkv[64 * h1 : 64 * h1 + 64])  # q of h1
            rows.append(W_qkv[512 + 64 * h0 : 512 + 64 * h0 + 64])  # k of h0
            rows.append(W_qkv[512 + 64 * h1 : 512 + 64 * h1 + 64])  # k of h1
        W_perm = np.concatenate(rows, axis=0)  # (512, 512)
        wqkT = np.ascontiguousarray(W_perm.T).astype(BF16)
        wv = W_qkv[1024 + 256 * g : 1024 + 256 * (g + 1)]  # (256, 512)
        wvT = np.ascontiguousarray(wv.T).astype(BF16)
        wo = W_out[:, 256 * g : 256 * (g + 1)]  # (512, 256)
        woT = np.ascontiguousarray(wo.T).astype(BF16)
        in_maps.append(
            {"xT": xT, "wqkT": wqkT, "wvT": wvT, "woT": woT, "triu2": triu2}
        )
    return in_maps


def kernel(x, W_qkv, W_out, b_out):
    from concourse.bass_utils import run_bass_kernel_spmd

    x = np.asarray(x, np.float32)
    W_qkv = np.asarray(W_qkv, np.float32)
    W_out = np.asarray(W_out, np.float32)
    b_out = np.asarray(b_out, np.float32)

    nc = _get_program()
    in_maps = _make_in_maps(x, W_qkv, W_out)
    res = run_bass_kernel_spmd(nc, in_maps, list(range(N_CORES)))
    outs = [r["out"].astype(np.float32) for r in res.results]
    full = np.empty((B, T, D), np.float32)
    for b in range(B):
        full[b] = outs[2 * b] + outs[2 * b + 1] + b_out
    return full


# revision 29
# speedup vs baseline: 1.8406x; 1.8406x over previous
"""Causal self-attention Bass kernel for Trainium2, 8 NeuronCores.

Problem shapes (hardcoded): x (4, 2048, 512), W_qkv (1536, 512),
W_out (512, 512), b_out (512,); NH=8 heads, DH=64.

Sharding: core c handles batch b = c // 2 and head group g = c % 2
(4 heads each). Each core computes its QKV slice, causal attention for
its 4 heads, and a partial output projection over its 256 y-dims.
Host sums the two partials per batch and adds the bias.

Per-core device pipeline (all matmul operands bf16, f32 PSUM accum):
  1. qkT projection: qkT[c_loc, t] chunks ordered [q-pair0, k-pair0,
     q-pair1, k-pair1] so each head's qT/kT share a 64-partition range.
  2. v projection directly in natural [t, d] layout, augmented with a
     ones column per head (denominator trick).
  3. Per head h, per key chunk j (128 rows): scoresT[k, q] for q >=
     128j only (causality via matmul geometry), one exp ACTIVATE per
     (h, j) with the 1/8 scale folded in, triangular mask applied to
     the 128x128 diagonal block post-exp (gpsimd multiply).
  4. att @ v with lhsT = [v | ones] (M=65): row 64 accumulates the
     softmax denominators for free.
  5. y normalization: partition-broadcast of the denominator row,
     vector reciprocal + multiply into the stacked ynormT layout.
  6. Output projection partial (K = 256 local y-dims).
"""

import sys

if "/opt/trn_rl_repo" not in sys.path:
    sys.path.insert(0, "/opt/trn_rl_repo")

import numpy as np
import ml_dtypes

B, T, D, NH, DH = 4, 2048, 512, 8, 64
HPC = 4  # heads per core
N_CORES = 8
BF16 = ml_dtypes.bfloat16

_PROG = None


def _build_program():
    import concourse.bass as bass
    import concourse.mybir as mybir
    import concourse.tile as tile
    from concourse import bacc

    f32 = mybir.dt.float32
    bf16 = mybir.dt.bfloat16
    Exp = mybir.ActivationFunctionType.Exp

    nc = bacc.Bacc("TRN2", target_bir_lowering=False, debug=False)

    xT_d = nc.dram_tensor("xT", [D, T], bf16, kind="ExternalInput").ap()
    wqkT_d = nc.dram_tensor("wqkT", [D, 512], bf16, kind="ExternalInput").ap()
    wvT_d = nc.dram_tensor("wvT", [D, 256], bf16, kind="ExternalInput").ap()
    woT_d = nc.dram_tensor("woT", [256, D], bf16, kind="ExternalInput").ap()
    triu_d = nc.dram_tensor("triu", [128, 128], bf16, kind="ExternalInput").ap()
    out_d = nc.dram_tensor("out", [T, D], bf16, kind="ExternalOutput").ap()

    NT = T // 128  # 16 key/query 128-chunks
    NQ = T // 512  # 4 query 512-chunks

    with tile.TileContext(nc) as tc:
        with (
            tc.tile_pool(name="const", bufs=1) as cpool,
            tc.tile_pool(name="att", bufs=2) as apool,
            tc.tile_pool(name="work", bufs=2) as wpool,
            tc.tile_pool(name="outsb", bufs=2) as opool,
            tc.tile_pool(name="ps", bufs=1, space="PSUM") as pspool,
        ):
            # ---- input DMAs ----
            xT_sb = cpool.tile([128, 4, T], bf16, tag="xT")
            for kc in range(4):
                nc.sync.dma_start(out=xT_sb[:, kc, :], in_=xT_d[128 * kc : 128 * (kc + 1), :])
            wqkT_sb = cpool.tile([128, 4, 512], bf16, tag="wqkT")
            for kc in range(4):
                nc.sync.dma_start(out=wqkT_sb[:, kc, :], in_=wqkT_d[128 * kc : 128 * (kc + 1), :])
            wvT_sb = cpool.tile([128, 4, 256], bf16, tag="wvT")
            for kc in range(4):
                nc.sync.dma_start(out=wvT_sb[:, kc, :], in_=wvT_d[128 * kc : 128 * (kc + 1), :])
            woT_sb = cpool.tile([128, 2, 512], bf16, tag="woT")
            for kc in range(2):
                nc.sync.dma_start(out=woT_sb[:, kc, :], in_=woT_d[128 * kc : 128 * (kc + 1), :])
            triu_sb = cpool.tile([128, 128], bf16, tag="triu")
            nc.sync.dma_start(out=triu_sb[:, :], in_=triu_d[:, :])

            # qkT chunks: [q-pair0 | k-pair0 | q-pair1 | k-pair1]; head h at
            # partitions 64*(h%2) .. +64 of chunks (2*(h//2), 2*(h//2)+1).
            qkT_sb = cpool.tile([128, 4, T], bf16, tag="qkT")
            # v_all: per t-chunk, per head: 64 v-dims + a ones column (65).
            v_all = cpool.tile([128, NT, HPC * 65], bf16, tag="v_all")
            ynormT = cpool.tile([128, 2, T], bf16, tag="ynormT")

            ones_ap = v_all[:, :, :].rearrange("p t (h e) -> p (t h) e", e=65)[:, :, 64:65]
            nc.gpsimd.memset(ones_ap, 1.0)
            # ones row at partition 64 used to broadcast the denominator row
            # via a rank-1 fp32 matmul (walrus can't codegen
            # InstPartitionBroadcast; matmul wants lhsT/rhs on the same base
            # partition as the den row, and fp32r inputs would need rounding)
            ones64 = cpool.tile([65, 64], f32, tag="ones64")
            nc.gpsimd.memset(ones64[:, :], 1.0)

            # ---- phase A: projections (interleave qk chunks with v halves
            # so the PSUM slots alternate and DVE copies hide under PE) ----
            def emit_proj_m(m):
                for half in range(2):
                    ps = pspool.tile([128, 1024], f32, tag="ps_s", bufs=2)
                    for n2 in range(2):
                        n = 2 * half + n2
                        for kc in range(4):
                            nc.tensor.matmul(
                                ps[:, 512 * n2 : 512 * (n2 + 1)],
                                lhsT=wqkT_sb[:, kc, 128 * m : 128 * (m + 1)],
                                rhs=xT_sb[:, kc, 512 * n : 512 * (n + 1)],
                                start=(kc == 0),
                                stop=(kc == 3),
                            )
                    nc.vector.tensor_copy(
                        qkT_sb[:, m, 1024 * half : 1024 * (half + 1)], ps[:, :]
                    )

            def emit_proj_v(half):
                ps = pspool.tile([128, T], f32, tag="ps_y")
                for tl in range(8):
                    t = 8 * half + tl
                    for kc in range(4):
                        nc.tensor.matmul(
                            ps[:, 256 * tl : 256 * (tl + 1)],
                            lhsT=xT_sb[:, kc, 128 * t : 128 * (t + 1)],
                            rhs=wvT_sb[:, kc, :],
                            start=(kc == 0),
                            stop=(kc == 3),
                        )
                dst = v_all[:, 8 * half : 8 * (half + 1), :].rearrange(
                    "p t (h e) -> p t h e", e=65
                )[:, :, :, 0:64]
                src = ps[:, :].rearrange("p (t h e) -> p t h e", t=8, h=HPC)
                nc.vector.tensor_copy(dst, src)

            emit_proj_m(0)
            emit_proj_v(0)
            emit_proj_m(1)
            emit_proj_v(1)
            emit_proj_m(2)
            emit_proj_m(3)

            # ---- phase B/C: per-head attention ----
            def emit_av(h, j, ps_y, att):
                # accumulate yT_aug[:, q] += v_augT @ attT for key chunk j
                q0 = 128 * j
                lhsT = v_all[:, j, :].rearrange("p (h e) -> p h e", e=65)[:, h, :]
                for c in range(NQ):
                    if j > 4 * c + 3:
                        continue  # fully masked: k > all q in this chunk
                    qs = max(512 * c, q0)
                    qe = 512 * (c + 1)
                    nc.tensor.matmul(
                        ps_y[:, qs:qe],
                        lhsT=lhsT,
                        rhs=att[:, qs - q0 : qe - q0],
                        start=(j == 0),
                        stop=(j == 4 * c + 3),
                    )

            for h in range(HPC):
                base = 64 * (h % 2)
                qc = 2 * (h // 2)  # q chunk index; k chunk is qc + 1
                qT = qkT_sb[base : base + 64, qc, :]
                kT = qkT_sb[base : base + 64, qc + 1, :]
                ps_y = pspool.tile([65, T], f32, tag="ps_y")
                att_tiles = {}
                for j in range(NT):
                    q0 = 128 * j
                    att = apool.tile([128, T - q0], bf16, tag=f"att{j}")
                    # split the q range at 1024 so scores(j+1) can run in the
                    # second ps_s buffer while exp(j) drains the first
                    halves = [(q0, 1024), (1024, T)] if j < 8 else [(q0, T)]
                    for hs, he in halves:
                        ps_s = pspool.tile([128, 1024], f32, tag="ps_s", bufs=2)
                        for rel in range(0, he - hs, 512):
                            n = min(512, he - hs - rel)
                            nc.tensor.matmul(
                                ps_s[:, rel : rel + n],
                                lhsT=kT[:, 128 * j : 128 * (j + 1)],
                                rhs=qT[:, hs + rel : hs + rel + n],
                                start=True,
                                stop=True,
                            )
                        nc.scalar.activation(
                            att[:, hs - q0 : he - q0],
                            ps_s[:, 0 : he - hs],
                            Exp,
                            scale=0.125,
                        )
                    # mask the strict upper triangle of the diagonal block
                    nc.gpsimd.tensor_mul(att[:, 0:128], att[:, 0:128], triu_sb[:, :])
                    att_tiles[j] = att
                    if j >= 2:
                        emit_av(h, j - 2, ps_y, att_tiles[j - 2])
                emit_av(h, NT - 2, ps_y, att_tiles[NT - 2])
                emit_av(h, NT - 1, ps_y, att_tiles[NT - 1])

                # normalization: y / den, written to stacked ynormT
                yT = wpool.tile([65, T], f32, tag="yT")
                nc.vector.tensor_copy(yT[:, :], ps_y[:, :])
                # broadcast den row across 64 partitions: ones[1,64].T @ den
                ps_b = pspool.tile([64, T], f32, tag="ps_y")
                for c in range(NQ):
                    nc.tensor.matmul(
                        ps_b[:, 512 * c : 512 * (c + 1)],
                        lhsT=ones64[64:65, :],
                        rhs=yT[64:65, 512 * c : 512 * (c + 1)],
                        start=True,
                        stop=True,
                    )
                recb = wpool.tile([64, T], f32, tag="recb")
                nc.vector.reciprocal_approx_fast(out=recb[:, :], in_=ps_b[:, :])
                dst = ynormT[base : base + 64, h // 2, :]
                nc.vector.tensor_mul(dst, yT[0:64, :], recb[:, :])

            # ---- phase D: output projection partial ----
            for grp in range(8):
                ps = pspool.tile([128, 1024], f32, tag="ps_s", bufs=2)
                for i2 in range(2):
                    i = 2 * grp + i2
                    for kc in range(2):
                        nc.tensor.matmul(
                            ps[:, 512 * i2 : 512 * (i2 + 1)],
                            lhsT=ynormT[:, kc, 128 * i : 128 * (i + 1)],
                            rhs=woT_sb[:, kc, :],
                            start=(kc == 0),
                            stop=(kc == 1),
                        )
                osb = opool.tile([128, 1024], bf16, tag="osb")
                nc.vector.tensor_copy(osb[:, :], ps[:, :])
                for i2 in range(2):
                    i = 2 * grp + i2
                    nc.sync.dma_start(
                        out=out_d[128 * i : 128 * (i + 1), :],
                        in_=osb[:, 512 * i2 : 512 * (i2 + 1)],
                    )

    nc.compile()
    return nc


def _get_program():
    global _PROG
    if _PROG is None:
        _PROG = _build_program()
    return _PROG


def _make_in_maps(x, W_qkv, W_out):
    in_maps = []
    triu = np.triu(np.ones((128, 128), np.float32)).astype(BF16)
    for c in range(N_CORES):
        b, g = c // 2, c % 2
        heads = [4 * g + i for i in range(HPC)]
        xT = np.ascontiguousarray(x[b].T).astype(BF16)
        # qkT chunk order: [q-pair0, k-pair0, q-pair1, k-pair1]
        rows = []
        for hp in range(2):
            h0, h1 = heads[2 * hp], heads[2 * hp + 1]
            rows.append(W_qkv[64 * h0 : 64 * h0 + 64])  # q of h0
            rows.append(W_qkv[64 * h1 : 64 * h1 + 64])  # q of h1
            rows.append(W_qkv[512 + 64 * h0 : 512 + 64 * h0 + 64])  # k of h0
            rows.append(W_qkv[512 + 64 * h1 : 512 + 64 * h1 + 64])  # k of h1
        W_perm = np.concatenate(rows, axis=0)  # (512, 512)
        wqkT = np.ascontiguousarray(W_perm.T).astype(BF16)
        wv = W_qkv[1024 + 256 * g : 1024 + 256 * (g + 1)]  # (256, 512)
        wvT = np.ascontiguousarray(wv.T).astype(BF16)
        wo = W_out[:, 256 * g : 256 * (g + 1)]  # (512, 256)
        woT = np.ascontiguousarray(wo.T).astype(BF16)
        in_maps.append(
            {"xT": xT, "wqkT": wqkT, "wvT": wvT, "woT": woT, "triu": triu}
        )
    return in_maps


def kernel(x, W_qkv, W_out, b_out):
    from concourse.bass_utils import run_bass_kernel_spmd

    x = np.asarray(x, np.float32)
    W_qkv = np.asarray(W_qkv, np.float32)
    W_out = np.asarray(W_out, np.float32)
    b_out = np.asarray(b_out, np.float32)

    nc = _get_program()
    in_maps = _make_in_maps(x, W_qkv, W_out)
    res = run_bass_kernel_spmd(nc, in_maps, list(range(N_CORES)))
    outs = [r["out"].astype(np.float32) for r in res.results]
    full = np.empty((B, T, D), np.float32)
    for b in range(B):
        full[b] = outs[2 * b] + outs[2 * b + 1] + b_out
    return full



# revision 32
# speedup vs baseline: 2.1612x; 1.1742x over previous
"""Causal self-attention Bass kernel for Trainium2, 8 NeuronCores.

Problem shapes (hardcoded): x (4, 2048, 512), W_qkv (1536, 512),
W_out (512, 512), b_out (512,); NH=8 heads, DH=64.

Sharding: core c handles batch b = c // 2 and head group g = c % 2
(4 heads each). Each core computes its QKV slice, causal attention for
its 4 heads, and a partial output projection over its 256 y-dims.
Host sums the two partials per batch and adds the bias.

Per-core device pipeline (all matmul operands bf16, f32 PSUM accum):
  1. qkT projection: qkT[c_loc, t] chunks ordered [q-pair0, k-pair0,
     q-pair1, k-pair1] so each head's qT/kT share a 64-partition range.
  2. v projection directly in natural [t, d] layout, augmented with a
     ones column per head (denominator trick).
  3. Per head h, per key chunk j (128 rows): scoresT[k, q] for q >=
     128j only (causality via matmul geometry), one exp ACTIVATE per
     (h, j) with the 1/8 scale folded in, triangular mask applied to
     the 128x128 diagonal block post-exp (gpsimd multiply).
  4. att @ v with lhsT = [v | ones] (M=65): row 64 accumulates the
     softmax denominators for free.
  5. y normalization: partition-broadcast of the denominator row,
     vector reciprocal + multiply into the stacked ynormT layout.
  6. Output projection partial (K = 256 local y-dims).
"""

import sys

if "/opt/trn_rl_repo" not in sys.path:
    sys.path.insert(0, "/opt/trn_rl_repo")

import numpy as np
import ml_dtypes

B, T, D, NH, DH = 4, 2048, 512, 8, 64
HPC = 4  # heads per core
N_CORES = 8
BF16 = ml_dtypes.bfloat16

_PROG = None


def _build_program():
    import concourse.bass as bass
    import concourse.mybir as mybir
    import concourse.tile as tile
    from concourse import bacc

    f32 = mybir.dt.float32
    bf16 = mybir.dt.bfloat16
    Exp = mybir.ActivationFunctionType.Exp

    nc = bacc.Bacc("TRN2", target_bir_lowering=False, debug=False)

    xT_d = nc.dram_tensor("xT", [D, T], bf16, kind="ExternalInput").ap()
    wqkT_d = nc.dram_tensor("wqkT", [D, 512], bf16, kind="ExternalInput").ap()
    wvT_d = nc.dram_tensor("wvT", [D, 256], bf16, kind="ExternalInput").ap()
    woT_d = nc.dram_tensor("woT", [256, D], bf16, kind="ExternalInput").ap()
    triu_d = nc.dram_tensor("triu", [128, 128], bf16, kind="ExternalInput").ap()
    out_d = nc.dram_tensor("out", [T, D], bf16, kind="ExternalOutput").ap()

    NT = T // 128  # 16 key/query 128-chunks
    NQ = T // 512  # 4 query 512-chunks

    with tile.TileContext(nc) as tc:
        with (
            tc.tile_pool(name="const", bufs=1) as cpool,
            tc.tile_pool(name="att", bufs=2) as apool,
            tc.tile_pool(name="work", bufs=2) as wpool,
            tc.tile_pool(name="outsb", bufs=2) as opool,
            tc.tile_pool(name="ps", bufs=1, space="PSUM") as pspool,
        ):
            # ---- input DMAs ----
            xT_sb = cpool.tile([128, 4, T], bf16, tag="xT")
            for kc in range(4):
                nc.sync.dma_start(out=xT_sb[:, kc, :], in_=xT_d[128 * kc : 128 * (kc + 1), :])
            wqkT_sb = cpool.tile([128, 4, 512], bf16, tag="wqkT")
            for kc in range(4):
                nc.sync.dma_start(out=wqkT_sb[:, kc, :], in_=wqkT_d[128 * kc : 128 * (kc + 1), :])
            wvT_sb = cpool.tile([128, 4, 256], bf16, tag="wvT")
            for kc in range(4):
                nc.sync.dma_start(out=wvT_sb[:, kc, :], in_=wvT_d[128 * kc : 128 * (kc + 1), :])
            woT_sb = cpool.tile([128, 2, 512], bf16, tag="woT")
            for kc in range(2):
                nc.sync.dma_start(out=woT_sb[:, kc, :], in_=woT_d[128 * kc : 128 * (kc + 1), :])
            triu_sb = cpool.tile([128, 128], bf16, tag="triu")
            nc.sync.dma_start(out=triu_sb[:, :], in_=triu_d[:, :])

            # qkT chunks: [q-pair0 | k-pair0 | q-pair1 | k-pair1]; head h at
            # partitions 64*(h%2) .. +64 of chunks (2*(h//2), 2*(h//2)+1).
            qkT_sb = cpool.tile([128, 4, T], bf16, tag="qkT")
            # v_all: per t-chunk, per head: 64 v-dims + a ones column (65).
            v_all = cpool.tile([128, NT, HPC * 65], bf16, tag="v_all")
            ynormT = cpool.tile([128, 2, T], bf16, tag="ynormT")

            ones_ap = v_all[:, :, :].rearrange("p t (h e) -> p (t h) e", e=65)[:, :, 64:65]
            nc.gpsimd.memset(ones_ap, 1.0)
            # ones row at partition 64 used to broadcast the denominator row
            # via a rank-1 fp32 matmul (walrus can't codegen
            # InstPartitionBroadcast; matmul wants lhsT/rhs on the same base
            # partition as the den row, and fp32r inputs would need rounding)
            ones64 = cpool.tile([65, 64], bf16, tag="ones64")
            nc.gpsimd.memset(ones64[:, :], 1.0)

            # ---- phase A: projections (interleave qk chunks with v halves
            # so the PSUM slots alternate and DVE copies hide under PE) ----
            def emit_proj_m(m):
                for half in range(2):
                    ps = pspool.tile([128, 1024], f32, tag="ps_s", bufs=2)
                    for n2 in range(2):
                        n = 2 * half + n2
                        for kc in range(4):
                            nc.tensor.matmul(
                                ps[:, 512 * n2 : 512 * (n2 + 1)],
                                lhsT=wqkT_sb[:, kc, 128 * m : 128 * (m + 1)],
                                rhs=xT_sb[:, kc, 512 * n : 512 * (n + 1)],
                                start=(kc == 0),
                                stop=(kc == 3),
                            )
                    nc.vector.tensor_copy(
                        qkT_sb[:, m, 1024 * half : 1024 * (half + 1)], ps[:, :]
                    )

            def emit_proj_v(half):
                ps = pspool.tile([128, T], f32, tag="ps_y")
                for tl in range(8):
                    t = 8 * half + tl
                    for kc in range(4):
                        nc.tensor.matmul(
                            ps[:, 256 * tl : 256 * (tl + 1)],
                            lhsT=xT_sb[:, kc, 128 * t : 128 * (t + 1)],
                            rhs=wvT_sb[:, kc, :],
                            start=(kc == 0),
                            stop=(kc == 3),
                        )
                dst = v_all[:, 8 * half : 8 * (half + 1), :].rearrange(
                    "p t (h e) -> p t h e", e=65
                )[:, :, :, 0:64]
                src = ps[:, :].rearrange("p (t h e) -> p t h e", t=8, h=HPC)
                nc.vector.tensor_copy(dst, src)

            emit_proj_m(0)
            emit_proj_v(0)
            emit_proj_m(1)
            emit_proj_v(1)
            emit_proj_m(2)
            emit_proj_m(3)

            # ---- phase B/C: per-head attention ----
            def emit_av(h, j, ps_y, att):
                # accumulate yT_aug[:, q] += v_augT @ attT for key chunk j
                q0 = 128 * j
                lhsT = v_all[:, j, :].rearrange("p (h e) -> p h e", e=65)[:, h, :]
                for c in range(NQ):
                    if j > 4 * c + 3:
                        continue  # fully masked: k > all q in this chunk
                    qs = max(512 * c, q0)
                    qe = 512 * (c + 1)
                    nc.tensor.matmul(
                        ps_y[:, qs:qe],
                        lhsT=lhsT,
                        rhs=att[:, qs - q0 : qe - q0],
                        start=(j == 0),
                        stop=(j == 4 * c + 3),
                    )

            def emit_norm_half(h, half, ps_y):
                """normalize y[:, 1024*half : +1024]; half 0 runs while
                avs of j >= 10 continue (column-disjoint). The den row is
                cast to bf16 so the rank-1 broadcast matmuls run at bf16
                rate (fp32 matmuls are 4x slower on the PE); the broadcast
                lands back in the already-copied-out region of ps_y."""
                base = 64 * (h % 2)
                c0, c1 = 1024 * half, 1024 * (half + 1)
                yT = wpool.tile([65, 1024], f32, tag="yT", name=f"yT{h}_{half}")
                nc.vector.tensor_copy(yT[:, :], ps_y[:, c0:c1])
                den = wpool.tile([1, 1024], bf16, tag="den", name=f"den{h}_{half}")
                nc.vector.tensor_copy(den[:, :], yT[64:65, :])
                for c2 in range(2):
                    nc.tensor.matmul(
                        ps_y[0:64, c0 + 512 * c2 : c0 + 512 * (c2 + 1)],
                        lhsT=ones64[0:1, :],
                        rhs=den[0:1, 512 * c2 : 512 * (c2 + 1)],
                        start=True,
                        stop=True,
                    )
                recb = wpool.tile([64, 1024], f32, tag="recb", name=f"recb{h}_{half}")
                nc.vector.reciprocal_approx_fast(out=recb[:, :], in_=ps_y[0:64, c0:c1])
                dst = ynormT[base : base + 64, h // 2, c0:c1]
                nc.vector.tensor_mul(dst, yT[0:64, :], recb[:, :])

            def emit_outproj(grp):
                ps = pspool.tile([128, 1024], f32, tag="ps_s", bufs=2)
                for i2 in range(2):
                    i = 2 * grp + i2
                    for kc in range(2):
                        nc.tensor.matmul(
                            ps[:, 512 * i2 : 512 * (i2 + 1)],
                            lhsT=ynormT[:, kc, 128 * i : 128 * (i + 1)],
                            rhs=woT_sb[:, kc, :],
                            start=(kc == 0),
                            stop=(kc == 1),
                        )
                osb = opool.tile([128, 1024], bf16, tag="osb")
                nc.vector.tensor_copy(osb[:, :], ps[:, :])
                for i2 in range(2):
                    i = 2 * grp + i2
                    nc.sync.dma_start(
                        out=out_d[128 * i : 128 * (i + 1), :],
                        in_=osb[:, 512 * i2 : 512 * (i2 + 1)],
                    )

            for h in range(HPC):
                base = 64 * (h % 2)
                qc = 2 * (h // 2)  # q chunk index; k chunk is qc + 1
                qT = qkT_sb[base : base + 64, qc, :]
                kT = qkT_sb[base : base + 64, qc + 1, :]
                ps_y = pspool.tile([65, T], f32, tag="ps_y")
                att_tiles = {}
                for j in range(NT):
                    q0 = 128 * j
                    att = apool.tile([128, T - q0], bf16, tag=f"att{j}")
                    # split the q range at 1024 so scores(j+1) can run in the
                    # second ps_s buffer while exp(j) drains the first
                    halves = [(q0, 1024), (1024, T)] if j < 8 else [(q0, T)]
                    for hs, he in halves:
                        ps_s = pspool.tile([128, 1024], f32, tag="ps_s", bufs=2)
                        for rel in range(0, he - hs, 512):
                            n = min(512, he - hs - rel)
                            nc.tensor.matmul(
                                ps_s[:, rel : rel + n],
                                lhsT=kT[:, 128 * j : 128 * (j + 1)],
                                rhs=qT[:, hs + rel : hs + rel + n],
                                start=True,
                                stop=True,
                            )
                        nc.scalar.activation(
                            att[:, hs - q0 : he - q0],
                            ps_s[:, 0 : he - hs],
                            Exp,
                            scale=0.125,
                        )
                    # mask the strict upper triangle of the diagonal block
                    nc.gpsimd.tensor_mul(att[:, 0:128], att[:, 0:128], triu_sb[:, :])
                    att_tiles[j] = att
                    if j >= 2:
                        emit_av(h, j - 2, ps_y, att_tiles[j - 2])
                    if j == 11:
                        # cols 0:1024 of ps_y complete (last write av(7),
                        # emitted at j==9); remaining avs touch cols >=1280
                        emit_norm_half(h, 0, ps_y)
                if h < HPC - 1:
                    emit_av(h, NT - 2, ps_y, att_tiles[NT - 2])
                    emit_av(h, NT - 1, ps_y, att_tiles[NT - 1])
                    emit_norm_half(h, 1, ps_y)
                else:
                    # interleave the half-0 output projections (all heads'
                    # half-0 ynormT is ready) with the final avs so the PE
                    # stays fed while the last norm chain drains
                    emit_av(h, NT - 2, ps_y, att_tiles[NT - 2])
                    emit_outproj(0)
                    emit_outproj(1)
                    emit_av(h, NT - 1, ps_y, att_tiles[NT - 1])
                    emit_outproj(2)
                    emit_outproj(3)
                    emit_norm_half(h, 1, ps_y)

            # ---- phase D: remaining output projection partials ----
            for grp in range(4, 8):
                emit_outproj(grp)

    nc.compile()
    return nc


def _get_program():
    global _PROG
    if _PROG is None:
        _PROG = _build_program()
    return _PROG


def _make_in_maps(x, W_qkv, W_out):
    in_maps = []
    triu = np.triu(np.ones((128, 128), np.float32)).astype(BF16)
    for c in range(N_CORES):
        b, g = c // 2, c % 2
        heads = [4 * g + i for i in range(HPC)]
        xT = np.ascontiguousarray(x[b].T).astype(BF16)
        # qkT chunk order: [q-pair0, k-pair0, q-pair1, k-pair1]
        rows = []
        for hp in range(2):
            h0, h1 = heads[2 * hp], heads[2 * hp + 1]
            rows.append(W_qkv[64 * h0 : 64 * h0 + 64])  # q of h0
            rows.append(W_qkv[64 * h1 : 64 * h1 + 64])  # q of h1
            rows.append(W_qkv[512 + 64 * h0 : 512 + 64 * h0 + 64])  # k of h0
            rows.append(W_qkv[512 + 64 * h1 : 512 + 64 * h1 + 64])  # k of h1
        W_perm = np.concatenate(rows, axis=0)  # (512, 512)
        wqkT = np.ascontiguousarray(W_perm.T).astype(BF16)
        wv = W_qkv[1024 + 256 * g : 1024 + 256 * (g + 1)]  # (256, 512)
        wvT = np.ascontiguousarray(wv.T).astype(BF16)
        wo = W_out[:, 256 * g : 256 * (g + 1)]  # (512, 256)
        woT = np.ascontiguousarray(wo.T).astype(BF16)
        in_maps.append(
            {"xT": xT, "wqkT": wqkT, "wvT": wvT, "woT": woT, "triu": triu}
        )
    return in_maps


def kernel(x, W_qkv, W_out, b_out):
    from concourse.bass_utils import run_bass_kernel_spmd

    x = np.asarray(x, np.float32)
    W_qkv = np.asarray(W_qkv, np.float32)
    W_out = np.asarray(W_out, np.float32)
    b_out = np.asarray(b_out, np.float32)

    nc = _get_program()
    in_maps = _make_in_maps(x, W_qkv, W_out)
    res = run_bass_kernel_spmd(nc, in_maps, list(range(N_CORES)))
    outs = [r["out"].astype(np.float32) for r in res.results]
    full = np.empty((B, T, D), np.float32)
    for b in range(B):
        full[b] = outs[2 * b] + outs[2 * b + 1] + b_out
    return full



# revision 34
# speedup vs baseline: 2.2201x; 1.0273x over previous
"""Causal self-attention Bass kernel for Trainium2, 8 NeuronCores.

Problem shapes (hardcoded): x (4, 2048, 512), W_qkv (1536, 512),
W_out (512, 512), b_out (512,); NH=8 heads, DH=64.

Sharding: core c handles batch b = c // 2 and head group g = c % 2
(4 heads each). Each core computes its QKV slice, causal attention for
its 4 heads, and a partial output projection over its 256 y-dims.
Host sums the two partials per batch and adds the bias.

Per-core device pipeline (all matmul operands bf16, f32 PSUM accum):
  1. qkT projection: qkT[c_loc, t] chunks ordered [q-pair0, k-pair0,
     q-pair1, k-pair1] so each head's qT/kT share a 64-partition range.
  2. v projection directly in natural [t, d] layout, augmented with a
     ones column per head (denominator trick).
  3. Per head h, per key chunk j (128 rows): scoresT[k, q] for q >=
     128j only (causality via matmul geometry), one exp ACTIVATE per
     (h, j) with the 1/8 scale folded in, triangular mask applied to
     the 128x128 diagonal block post-exp (gpsimd multiply).
  4. att @ v with lhsT = [v | ones] (M=65): row 64 accumulates the
     softmax denominators for free.
  5. y normalization: partition-broadcast of the denominator row,
     vector reciprocal + multiply into the stacked ynormT layout.
  6. Output projection partial (K = 256 local y-dims).
"""

import sys

if "/opt/trn_rl_repo" not in sys.path:
    sys.path.insert(0, "/opt/trn_rl_repo")

import numpy as np
import ml_dtypes

B, T, D, NH, DH = 4, 2048, 512, 8, 64
HPC = 4  # heads per core
N_CORES = 8
BF16 = ml_dtypes.bfloat16

_PROG = None


def _build_program():
    import concourse.bass as bass
    import concourse.mybir as mybir
    import concourse.tile as tile
    from concourse import bacc

    f32 = mybir.dt.float32
    bf16 = mybir.dt.bfloat16
    Exp = mybir.ActivationFunctionType.Exp

    nc = bacc.Bacc("TRN2", target_bir_lowering=False, debug=False)

    xT_d = nc.dram_tensor("xT", [D, T], bf16, kind="ExternalInput").ap()
    wqkT_d = nc.dram_tensor("wqkT", [D, 512], bf16, kind="ExternalInput").ap()
    wvT_d = nc.dram_tensor("wvT", [D, 256], bf16, kind="ExternalInput").ap()
    woT_d = nc.dram_tensor("woT", [256, D], bf16, kind="ExternalInput").ap()
    triu_d = nc.dram_tensor("triu", [128, 128], bf16, kind="ExternalInput").ap()
    out_d = nc.dram_tensor("out", [T, D], bf16, kind="ExternalOutput").ap()

    NT = T // 128  # 16 key/query 128-chunks
    NQ = T // 512  # 4 query 512-chunks

    with tile.TileContext(nc) as tc:
        with (
            tc.tile_pool(name="const", bufs=1) as cpool,
            tc.tile_pool(name="att", bufs=2) as apool,
            tc.tile_pool(name="work", bufs=2) as wpool,
            tc.tile_pool(name="outsb", bufs=2) as opool,
            tc.tile_pool(name="ps", bufs=1, space="PSUM") as pspool,
        ):
            # ---- input DMAs ----
            xT_sb = cpool.tile([128, 4, T], bf16, tag="xT")
            for kc in range(4):
                nc.sync.dma_start(out=xT_sb[:, kc, :], in_=xT_d[128 * kc : 128 * (kc + 1), :])
            wqkT_sb = cpool.tile([128, 4, 512], bf16, tag="wqkT")
            for kc in range(4):
                nc.sync.dma_start(out=wqkT_sb[:, kc, :], in_=wqkT_d[128 * kc : 128 * (kc + 1), :])
            wvT_sb = cpool.tile([128, 4, 256], bf16, tag="wvT")
            for kc in range(4):
                nc.sync.dma_start(out=wvT_sb[:, kc, :], in_=wvT_d[128 * kc : 128 * (kc + 1), :])
            woT_sb = cpool.tile([128, 2, 512], bf16, tag="woT")
            for kc in range(2):
                nc.sync.dma_start(out=woT_sb[:, kc, :], in_=woT_d[128 * kc : 128 * (kc + 1), :])
            triu_sb = cpool.tile([128, 128], bf16, tag="triu")
            nc.sync.dma_start(out=triu_sb[:, :], in_=triu_d[:, :])

            # qkT chunks: [q-pair0 | k-pair0 | q-pair1 | k-pair1]; head h at
            # partitions 64*(h%2) .. +64 of chunks (2*(h//2), 2*(h//2)+1).
            qkT_sb = cpool.tile([128, 4, T], bf16, tag="qkT")
            # v_all: per t-chunk, per head: 64 v-dims + a ones column (65).
            v_all = cpool.tile([128, NT, HPC * 65], bf16, tag="v_all")
            ynormT = cpool.tile([128, 2, T], bf16, tag="ynormT")

            ones_ap = v_all[:, :, :].rearrange("p t (h e) -> p (t h) e", e=65)[:, :, 64:65]
            nc.gpsimd.memset(ones_ap, 1.0)
            # ones row at partition 64 used to broadcast the denominator row
            # via a rank-1 fp32 matmul (walrus can't codegen
            # InstPartitionBroadcast; matmul wants lhsT/rhs on the same base
            # partition as the den row, and fp32r inputs would need rounding)
            ones64 = cpool.tile([65, 64], bf16, tag="ones64")
            nc.gpsimd.memset(ones64[:, :], 1.0)

            # ---- phase A: projections. Only m=0,1 (heads 0/1 qk) and the
            # first v half run up front; the rest interleave into head 0's
            # attention as PE filler (they use ps_s slots, never ps_y).
            def emit_proj_m_half(m, half):
                ps = pspool.tile([128, 1024], f32, tag="ps_s", bufs=2)
                for n2 in range(2):
                    n = 2 * half + n2
                    for kc in range(4):
                        nc.tensor.matmul(
                            ps[:, 512 * n2 : 512 * (n2 + 1)],
                            lhsT=wqkT_sb[:, kc, 128 * m : 128 * (m + 1)],
                            rhs=xT_sb[:, kc, 512 * n : 512 * (n + 1)],
                            start=(kc == 0),
                            stop=(kc == 3),
                        )
                nc.vector.tensor_copy(
                    qkT_sb[:, m, 1024 * half : 1024 * (half + 1)], ps[:, :]
                )

            def emit_proj_v_quarter(qt):
                ps = pspool.tile([128, 1024], f32, tag="ps_s", bufs=2)
                for tl in range(4):
                    t = 4 * qt + tl
                    for kc in range(4):
                        nc.tensor.matmul(
                            ps[:, 256 * tl : 256 * (tl + 1)],
                            lhsT=xT_sb[:, kc, 128 * t : 128 * (t + 1)],
                            rhs=wvT_sb[:, kc, :],
                            start=(kc == 0),
                            stop=(kc == 3),
                        )
                dst = v_all[:, 4 * qt : 4 * (qt + 1), :].rearrange(
                    "p t (h e) -> p t h e", e=65
                )[:, :, :, 0:64]
                src = ps[:, :].rearrange("p (t h e) -> p t h e", t=4, h=HPC)
                nc.vector.tensor_copy(dst, src)

            emit_proj_m_half(0, 0)
            emit_proj_v_quarter(0)
            emit_proj_m_half(0, 1)
            emit_proj_v_quarter(1)
            emit_proj_m_half(1, 0)
            emit_proj_m_half(1, 1)

            # ---- phase B/C: per-head attention ----
            def emit_av(h, j, ps_y, att):
                # accumulate yT_aug[:, q] += v_augT @ attT for key chunk j
                q0 = 128 * j
                lhsT = v_all[:, j, :].rearrange("p (h e) -> p h e", e=65)[:, h, :]
                for c in range(NQ):
                    if j > 4 * c + 3:
                        continue  # fully masked: k > all q in this chunk
                    qs = max(512 * c, q0)
                    qe = 512 * (c + 1)
                    nc.tensor.matmul(
                        ps_y[:, qs:qe],
                        lhsT=lhsT,
                        rhs=att[:, qs - q0 : qe - q0],
                        start=(j == 0),
                        stop=(j == 4 * c + 3),
                    )

            def emit_norm_half(h, half, ps_y):
                """normalize y[:, 1024*half : +1024]; half 0 runs while
                avs of j >= 10 continue (column-disjoint). The den row is
                cast to bf16 so the rank-1 broadcast matmuls run at bf16
                rate (fp32 matmuls are 4x slower on the PE); the broadcast
                lands back in the already-copied-out region of ps_y."""
                base = 64 * (h % 2)
                c0, c1 = 1024 * half, 1024 * (half + 1)
                yT = wpool.tile([65, 1024], f32, tag="yT", name=f"yT{h}_{half}")
                nc.vector.tensor_copy(yT[:, :], ps_y[:, c0:c1])
                den = wpool.tile([1, 1024], bf16, tag="den", name=f"den{h}_{half}")
                nc.vector.tensor_copy(den[:, :], yT[64:65, :])
                for c2 in range(2):
                    nc.tensor.matmul(
                        ps_y[0:64, c0 + 512 * c2 : c0 + 512 * (c2 + 1)],
                        lhsT=ones64[0:1, :],
                        rhs=den[0:1, 512 * c2 : 512 * (c2 + 1)],
                        start=True,
                        stop=True,
                    )
                recb = wpool.tile([64, 1024], f32, tag="recb", name=f"recb{h}_{half}")
                nc.vector.reciprocal_approx_fast(out=recb[:, :], in_=ps_y[0:64, c0:c1])
                dst = ynormT[base : base + 64, h // 2, c0:c1]
                nc.vector.tensor_mul(dst, yT[0:64, :], recb[:, :])

            def emit_outproj(grp):
                ps = pspool.tile([128, 1024], f32, tag="ps_s", bufs=2)
                for i2 in range(2):
                    i = 2 * grp + i2
                    for kc in range(2):
                        nc.tensor.matmul(
                            ps[:, 512 * i2 : 512 * (i2 + 1)],
                            lhsT=ynormT[:, kc, 128 * i : 128 * (i + 1)],
                            rhs=woT_sb[:, kc, :],
                            start=(kc == 0),
                            stop=(kc == 1),
                        )
                osb = opool.tile([128, 1024], bf16, tag="osb")
                nc.vector.tensor_copy(osb[:, :], ps[:, :])
                for i2 in range(2):
                    i = 2 * grp + i2
                    nc.sync.dma_start(
                        out=out_d[128 * i : 128 * (i + 1), :],
                        in_=osb[:, 512 * i2 : 512 * (i2 + 1)],
                    )

            for h in range(HPC):
                base = 64 * (h % 2)
                qc = 2 * (h // 2)  # q chunk index; k chunk is qc + 1
                qT = qkT_sb[base : base + 64, qc, :]
                kT = qkT_sb[base : base + 64, qc + 1, :]
                ps_y = pspool.tile([65, T], f32, tag="ps_y")
                att_tiles = {}
                for j in range(NT):
                    q0 = 128 * j
                    att = apool.tile([128, T - q0], bf16, tag=f"att{j}")
                    # split the q range at 1024 so scores(j+1) can run in the
                    # second ps_s buffer while exp(j) drains the first
                    halves = [(q0, 1024), (1024, T)] if j < 8 else [(q0, T)]
                    for hs, he in halves:
                        ps_s = pspool.tile([128, 1024], f32, tag="ps_s", bufs=2)
                        for rel in range(0, he - hs, 512):
                            n = min(512, he - hs - rel)
                            nc.tensor.matmul(
                                ps_s[:, rel : rel + n],
                                lhsT=kT[:, 128 * j : 128 * (j + 1)],
                                rhs=qT[:, hs + rel : hs + rel + n],
                                start=True,
                                stop=True,
                            )
                        nc.scalar.activation(
                            att[:, hs - q0 : he - q0],
                            ps_s[:, 0 : he - hs],
                            Exp,
                            scale=0.125,
                        )
                    # mask the strict upper triangle of the diagonal block
                    nc.gpsimd.tensor_mul(att[:, 0:128], att[:, 0:128], triu_sb[:, :])
                    att_tiles[j] = att
                    if j >= 2:
                        emit_av(h, j - 2, ps_y, att_tiles[j - 2])
                    if h == 0:
                        # deferred projections as PE filler for the
                        # scores->exp ping-pong gaps of the first head
                        if j == 2:
                            emit_proj_m_half(2, 0)
                        elif j == 3:
                            emit_proj_m_half(2, 1)
                        elif j == 4:
                            emit_proj_m_half(3, 0)
                        elif j == 5:
                            emit_proj_m_half(3, 1)
                        elif j == 6:
                            emit_proj_v_quarter(2)
                        elif j == 7:
                            emit_proj_v_quarter(3)
                    if j == 11:
                        # cols 0:1024 of ps_y complete (last write av(7),
                        # emitted at j==9); remaining avs touch cols >=1280
                        emit_norm_half(h, 0, ps_y)
                if h < HPC - 1:
                    emit_av(h, NT - 2, ps_y, att_tiles[NT - 2])
                    emit_av(h, NT - 1, ps_y, att_tiles[NT - 1])
                    emit_norm_half(h, 1, ps_y)
                else:
                    # interleave the half-0 output projections (all heads'
                    # half-0 ynormT is ready) with the final avs so the PE
                    # stays fed while the last norm chain drains
                    emit_av(h, NT - 2, ps_y, att_tiles[NT - 2])
                    emit_outproj(0)
                    emit_outproj(1)
                    emit_av(h, NT - 1, ps_y, att_tiles[NT - 1])
                    emit_outproj(2)
                    emit_outproj(3)
                    emit_norm_half(h, 1, ps_y)

            # ---- phase D: remaining output projection partials ----
            for grp in range(4, 8):
                emit_outproj(grp)

    nc.compile()
    return nc


def _get_program():
    global _PROG
    if _PROG is None:
        _PROG = _build_program()
    return _PROG


def _make_in_maps(x, W_qkv, W_out):
    in_maps = []
    triu = np.triu(np.ones((128, 128), np.float32)).astype(BF16)
    for c in range(N_CORES):
        b, g = c // 2, c % 2
        heads = [4 * g + i for i in range(HPC)]
        xT = np.ascontiguousarray(x[b].T).astype(BF16)
        # qkT chunk order: [q-pair0, k-pair0, q-pair1, k-pair1]
        rows = []
        for hp in range(2):
            h0, h1 = heads[2 * hp], heads[2 * hp + 1]
            rows.append(W_qkv[64 * h0 : 64 * h0 + 64])  # q of h0
            rows.append(W_qkv[64 * h1 : 64 * h1 + 64])  # q of h1
            rows.append(W_qkv[512 + 64 * h0 : 512 + 64 * h0 + 64])  # k of h0
            rows.append(W_qkv[512 + 64 * h1 : 512 + 64 * h1 + 64])  # k of h1
        W_perm = np.concatenate(rows, axis=0)  # (512, 512)
        wqkT = np.ascontiguousarray(W_perm.T).astype(BF16)
        wv = W_qkv[1024 + 256 * g : 1024 + 256 * (g + 1)]  # (256, 512)
        wvT = np.ascontiguousarray(wv.T).astype(BF16)
        wo = W_out[:, 256 * g : 256 * (g + 1)]  # (512, 256)
        woT = np.ascontiguousarray(wo.T).astype(BF16)
        in_maps.append(
            {"xT": xT, "wqkT": wqkT, "wvT": wvT, "woT": woT, "triu": triu}
        )
    return in_maps


def kernel(x, W_qkv, W_out, b_out):
    from concourse.bass_utils import run_bass_kernel_spmd

    x = np.asarray(x, np.float32)
    W_qkv = np.asarray(W_qkv, np.float32)
    W_out = np.asarray(W_out, np.float32)
    b_out = np.asarray(b_out, np.float32)

    nc = _get_program()
    in_maps = _make_in_maps(x, W_qkv, W_out)
    res = run_bass_kernel_spmd(nc, in_maps, list(range(N_CORES)))
    outs = [r["out"].astype(np.float32) for r in res.results]
    full = np.empty((B, T, D), np.float32)
    for b in range(B):
        full[b] = outs[2 * b] + outs[2 * b + 1] + b_out
    return full

